# revision 7
# baseline (speedup 1.0000x reference)
"""Trainium2 Bass kernel for nn_MetaNetLinearizedModel (v2).

Math (reference):
    xflat = x.reshape(B, D_IN)
    z1   = xflat @ W1.T + b1               # [B, FEAT]
    h    = relu(z1); base = h @ W2.T + b2  # [B, FEAT]
    coefs = relu(base @ mW1.T + mb1) @ mW2.T + mb2       # [B, T]
    u_t  = xflat @ dW1[t].T + db1[t]       # [B, FEAT]  (JVP of z1)
    tangent_t = (z1>0)*u_t @ W2.T + h @ dW2[t].T + db2[t]
    out  = base + sum_t coefs[:,t,None] * tangent_t

Memory traffic dominates (W1 bf16 77MB + dW1 fp8 154MB streamed over the
D_IN=150528 contraction).  D_IN is sharded 8 ways; per core ~30MB streams at
the ~360GB/s per-core DMA roofline (~84us floor).

v2 structural changes vs the first-session kernel:
  * W1 rides as bf16 only (no fp8 residual lane): rel_fro ~2.5e-3, 8x under
    the 2e-2 gate, saves 4.8MB/core of stream traffic.
  * Streams are ordered W1 first, then dW1.  The z partial finishes at ~1/3
    of the stream; its 8KB AllReduce plus the ENTIRE nonlinear tail
    (base/coefs/e-blocks) hide under the remaining dW1 streaming.
  * The u tangents are never AllReduced.  Each core applies mask, coefs and
    W2 to its LOCAL partial u (linearity of the tangent in u), adds 1/8 of
    the replicated base+e part, and writes its [B, FEAT] output PARTIAL;
    the host-side unshard (the gather step of the kernel() contract) sums
    the 8 partials.  Measured dependent-collective latencies here (8KB:
    AllReduce ~34.5us, AllGather ~17.5us) made any device-side final
    collective the single largest exposed cost; the only collective kept
    is the z AllReduce, which hides under the dW1 stream with ~30us slack.
  * Exposed endgame is ~5us: the last fp8 matmuls, a 4-op PSUM-direct
    coefficient combine (u biases pre-folded via a hidden-time PE matmul),
    mask, two transposes, the W2 projection, and the 8KB output DMA.
  * Weight streams are staged host-side in SBUF-native [128, NK, cols]
    layout so every DMA descriptor is a multi-KB contiguous partition line.

PE: x chunks [128,B] stationary; z-hi/z-lo/u01/u23 run concurrently in four
32-wide PE column groups (tile_position) so matmul wall ~ the longest lane.
"""

from contextlib import nullcontext

import numpy as np
import ml_dtypes

import concourse.bass as bass
import concourse.mybir as mybir
import concourse.tile as tile
from concourse import bacc
from concourse.bass_utils import run_bass_kernel_spmd

BF16 = ml_dtypes.bfloat16
FP8 = ml_dtypes.float8_e4m3

N_CORES = 8
B = 8
D_IN = 3 * 224 * 224      # 150528
FEAT = 256
HID = 64
T = 4
KC = D_IN // N_CORES      # 18816 per core
NK = KC // 128            # 147 k-chunks of 128
GROUP = 21                # k-chunks per weight DMA group (147 = 7 * 21)
DMA_SUB = 3               # sub-DMAs per group (must divide GROUP)
NGROUPS = NK // GROUP
DW1_SCALE = 256.0         # dW1 pre-scale so fp8e4m3 stays in normal range
UCOLS = T * FEAT          # 1024

F32 = mybir.dt.float32
BF = mybir.dt.bfloat16
F8 = mybir.dt.float8e4
AOT = mybir.AluOpType

_CACHE = {}

# bias row layout: [b2 | mb1 | mb2 | db2_0..3] (FEAT + HID + T + T*FEAT)
BROW_N = FEAT + HID + T + T * FEAT


def _emit_w1_stream(nc, tc, env, qs):
    """Stream W1 groups and run the bf16 z lane.

    Whole-group DMAs: 128 descriptors of 10.5KB contiguous per partition
    (descriptor size, not count, measured as the DMA-rate driver here);
    alternate split-halves across the two queues to keep load balanced."""
    wbf_d, wpool, xhi, bkzh = env
    for g in range(NGROUPS):
        wb = wpool.tile([128, GROUP, FEAT], BF, tag="wb", name="wb")
        half = GROUP // 3
        qs[g % 2].dma_start(wb[:, 0:half, :], wbf_d[:, g * GROUP:g * GROUP + half, :])
        qs[(g + 1) % 2].dma_start(wb[:, half:, :],
                                  wbf_d[:, g * GROUP + half:(g + 1) * GROUP, :])
        for c in range(GROUP):
            k = g * GROUP + c
            st = (k == 0)
            sp = (k == NK - 1)
            nc.tensor.matmul(bkzh[0:B, 0:FEAT], xhi[:, k, :],
                             wb[:, c, :], start=st, stop=sp,
                             tile_position=(0, 0))


def _emit_dw1_group(nc, tc, env, qs, g):
    """Stream one dW1 group and run the two fp8 u lanes.

    Groups 0..5: two half-group DMAs (one per queue, ~29KB/partition
    total).  Last group: 7 fine subs so the final matmuls and the exposed
    endgame start as early as possible."""
    w8_d, wpool8, xhi, bku01, bku23 = env
    w8 = wpool8.tile([128, GROUP, UCOLS], F8, tag="w8", name="w8")
    if g == NGROUPS - 1:
        # 3 subs of 7 chunks: 7.2KB descriptor lines stay in the fast DMA
        # regime (3KB lines measured ~45% slower) while the final matmuls
        # still start ~2us before the group's last byte lands.
        step = GROUP // 3
        for s in range(3):
            cs = slice(s * step, (s + 1) * step)
            gcs = slice(g * GROUP + cs.start, g * GROUP + cs.stop)
            q = qs[s % len(qs)]
            q.dma_start(w8[:, cs, :], w8_d[:, gcs, :])
    else:
        half = GROUP // 3
        qs[g % 2].dma_start(w8[:, 0:half, :],
                            w8_d[:, g * GROUP:g * GROUP + half, :])
        qs[(g + 1) % 2].dma_start(w8[:, half:, :],
                                  w8_d[:, g * GROUP + half:(g + 1) * GROUP, :])
    for c in range(GROUP):
        k = g * GROUP + c
        st = (k == 0)
        sp = (k == NK - 1)
        xh = xhi[:, k, :]
        nc.tensor.matmul(bku01[64:64 + B, :], xh, w8[:, c, 0:512],
                         start=st, stop=sp, tile_position=(0, 64))
        nc.tensor.matmul(bku23[96:96 + B, :], xh, w8[:, c, 512:1024],
                         start=st, stop=sp, tile_position=(0, 96))


def _emit_tail(nc, tc, env):
    """Nonlinear tail from the reduced z; replicated on every core.

    Produces (into env-out dict): h-derived mask, coefs, and
    q8 = (base + sum_t c_t e_t) / 8.  All of this runs hidden under the
    dW1 streaming: PE work interleaves into the stream's spare cycles.
    """
    (sb, sb2, ps_tp, ps2, ps_e, R_z, w2tb, mw1tb, dw2, mw2tb, browb,
     id8, ones1b, db18) = env

    h = sb.tile([B, FEAT], F32, tag="h", name="h")
    nc.vector.tensor_scalar_max(h[:], R_z[:], 0.0)
    mask = sb.tile([B, FEAT], F32, tag="mask", name="mask")
    nc.vector.tensor_scalar(mask[:], R_z[:], 0.0, None, op0=AOT.is_gt)

    def tr2b(dst, src):
        # src [B, 256] f32 -> dst sbuf [128, 2, B] bf16
        for c in range(2):
            tp = ps_tp.tile([128, B], F32, tag="tp", name="tp")
            nc.tensor.transpose(tp[:], src[:, c * 128:(c + 1) * 128],
                                id8[:])
            nc.vector.tensor_copy(dst[:, c, :], tp[:])

    hTb = sb.tile([128, 2, B], BF, tag="hTb", name="hTb")
    tr2b(hTb, h[:])

    pb = ps2.tile([B, FEAT], F32, tag="pp", name="pb")
    nc.tensor.matmul(pb[:], hTb[:, 0, :], w2tb[:, 0, :],
                     start=True, stop=False)
    nc.tensor.matmul(pb[:], hTb[:, 1, :], w2tb[:, 1, :],
                     start=False, stop=False)
    nc.tensor.matmul(pb[:], ones1b[:], browb[:, 0:FEAT],
                     start=False, stop=True)
    base = sb.tile([B, FEAT], F32, tag="base", name="base")
    nc.vector.tensor_copy(base[:], pb[:])

    bTb = sb.tile([128, 2, B], BF, tag="bTb", name="bTb")
    tr2b(bTb, base[:])
    pm = ps2.tile([B, HID], F32, tag="pp", name="pm")
    nc.tensor.matmul(pm[:], bTb[:, 0, :], mw1tb[:, 0, :],
                     start=True, stop=False)
    nc.tensor.matmul(pm[:], bTb[:, 1, :], mw1tb[:, 1, :],
                     start=False, stop=False)
    nc.tensor.matmul(pm[:], ones1b[:], browb[:, FEAT:FEAT + HID],
                     start=False, stop=True)
    m1 = sb.tile([B, HID], F32, tag="m1", name="m1")
    nc.vector.tensor_scalar_max(m1[:], pm[:], 0.0)

    tpm = ps_tp.tile([128, B], F32, tag="tp", name="tpm")
    nc.tensor.transpose(tpm[0:HID, :], m1[:], id8[:])
    m1tb = sb.tile([HID, B], BF, tag="m1tb", name="m1tb")
    nc.vector.tensor_copy(m1tb[:], tpm[0:HID, :])

    pc = ps2.tile([B, T], F32, tag="pp", name="pc")
    nc.tensor.matmul(pc[:], m1tb[:], mw2tb[:], start=True, stop=False)
    nc.tensor.matmul(pc[:], ones1b[:],
                     browb[:, FEAT + HID:FEAT + HID + T],
                     start=False, stop=True)
    coefs = sb.tile([B, T], F32, tag="coefs", name="coefs")
    nc.vector.tensor_copy(coefs[:], pc[:])

    # e blocks: h @ dW2_t.T (+ db2_t later), two tasks per PSUM bank;
    # dW2 rides fp8 (x256) so the STT combine rescales by 1/256.
    # db2 is folded in at the combine step (browb slice via base... kept
    # simple: db2 contributes c_t*db2_t, added via a separate ones-matmul
    # into the SAME bank pre-scaled by 256 on host is avoided; instead db2
    # joins through bias row matmul below with scale 256 applied host-side.
    pe1 = ps_e.tile([B, 512], F32, tag="pe", name="pe1")
    nc.tensor.matmul(pe1[:], hTb[:, 0, :], dw2[:, 0, 0:512],
                     start=True, stop=False)
    nc.tensor.matmul(pe1[:], hTb[:, 1, :], dw2[:, 1, 0:512],
                     start=False, stop=False)
    nc.tensor.matmul(pe1[:], ones1b[:],
                     browb[:, FEAT + HID + T:FEAT + HID + T + 512],
                     start=False, stop=True)
    pe2 = ps_e.tile([B, 512], F32, tag="pe", name="pe2")
    nc.tensor.matmul(pe2[:], hTb[:, 0, :], dw2[:, 0, 512:1024],
                     start=True, stop=False)
    nc.tensor.matmul(pe2[:], hTb[:, 1, :], dw2[:, 1, 512:1024],
                     start=False, stop=False)
    nc.tensor.matmul(pe2[:], ones1b[:],
                     browb[:, FEAT + HID + T + 512:BROW_N],
                     start=False, stop=True)

    # q8 = (base + sum_t c_t e_t) / 8   (replicated part, pre-scaled for AR)
    # e banks carry 256*(h@dW2_t.T + db2_t), so coefs are pre-divided.
    cq = sb.tile([B, T], F32, tag="cq", name="cq")
    nc.vector.tensor_scalar(cq[:], coefs[:], 1.0 / DW1_SCALE, None,
                            op0=AOT.mult)

    # cb = sum_t c_t * db1_t / 8  (hidden-time PE matmul over T partitions)
    tpc = ps_tp.tile([128, B], F32, tag="tp", name="tpc")
    nc.tensor.transpose(tpc[0:T, :], coefs[:], id8[:])
    coefsTb = sb.tile([T, B], BF, tag="coefsTb", name="coefsTb")
    nc.vector.tensor_copy(coefsTb[:], tpc[0:T, :])
    pcb = ps2.tile([B, FEAT], F32, tag="pp", name="pcb")
    nc.tensor.matmul(pcb[:], coefsTb[:], db18[:], start=True, stop=True)
    cb = sb.tile([B, FEAT], F32, tag="cb", name="cb")
    nc.vector.tensor_copy(cb[:], pcb[:])
    o = sb2.tile([B, FEAT], F32, tag="oacc", name="o0")
    nc.vector.scalar_tensor_tensor(o[:], pe1[:, 0:256], cq[:, 0:1],
                                   base[:], op0=AOT.mult, op1=AOT.add)
    o2 = sb2.tile([B, FEAT], F32, tag="oacc", name="o1")
    nc.vector.scalar_tensor_tensor(o2[:], pe1[:, 256:512], cq[:, 1:2],
                                   o[:], op0=AOT.mult, op1=AOT.add)
    o3 = sb2.tile([B, FEAT], F32, tag="oacc", name="o2")
    nc.vector.scalar_tensor_tensor(o3[:], pe2[:, 0:256],
                                   cq[:, 2:3], o2[:],
                                   op0=AOT.mult, op1=AOT.add)
    o4 = sb2.tile([B, FEAT], F32, tag="oacc", name="o3")
    nc.vector.scalar_tensor_tensor(o4[:], pe2[:, 256:512],
                                   cq[:, 3:4], o3[:],
                                   op0=AOT.mult, op1=AOT.add)
    q8 = sb.tile([B, FEAT], F32, tag="q8", name="q8")
    nc.vector.tensor_scalar(q8[:], o4[:], 1.0 / N_CORES, None, op0=AOT.mult)
    return mask, cq, cb, q8


def _build(reps1=1, body=1, wbufs=3):
    """Build the kernel.

    reps1 > 1 builds the BENCH variant: the stream portion (no collectives,
    no tail) wrapped in a dynamic For_i loop, statically duplicated `body`
    times, for slope-based device timing.  reps1 == 1 builds the production
    kernel with the full hidden-tail structure.
    """
    nc = bacc.Bacc("TRN2", target_bir_lowering=False, debug=False,
                   num_devices=N_CORES)

    wbf_d = nc.dram_tensor("wbf", [128, NK, FEAT], BF, kind="ExternalInput")
    w8_d = nc.dram_tensor("w8", [128, NK, UCOLS], F8, kind="ExternalInput")
    xhi_d = nc.dram_tensor("xhi", [128, NK, B], BF, kind="ExternalInput")
    w2tb_d = nc.dram_tensor("w2tb", [FEAT, FEAT], BF, kind="ExternalInput")
    mw1tb_d = nc.dram_tensor("mw1tb", [FEAT, HID], BF, kind="ExternalInput")
    dw2_d = nc.dram_tensor("dw2cat", [FEAT, T * FEAT], F8,
                           kind="ExternalInput")
    mw2tb_d = nc.dram_tensor("mw2tb", [HID, T], BF, kind="ExternalInput")
    browb_d = nc.dram_tensor("browb", [1, BROW_N], BF, kind="ExternalInput")
    biasz8_d = nc.dram_tensor("biasz8", [B, FEAT], F32, kind="ExternalInput")
    bias8u_d = nc.dram_tensor("bias8u", [B, UCOLS], F32, kind="ExternalInput")
    id8_d = nc.dram_tensor("ident8", [B, B], F32, kind="ExternalInput")
    db18_d = nc.dram_tensor("db18", [T, FEAT], BF, kind="ExternalInput")
    idsum_d = nc.dram_tensor("idsum", [N_CORES * B, B], F32,
                             kind="ExternalInput")
    out_d = nc.dram_tensor("out", [B, FEAT], F32, kind="ExternalOutput")

    with tile.TileContext(nc) as tc:
        with (
            tc.tile_pool(name="const", bufs=1) as cpool,
            tc.tile_pool(name="wstream", bufs=3) as wpool,
            tc.tile_pool(name="wstream8", bufs=wbufs) as wpool8,
            tc.tile_pool(name="sb", bufs=1) as sb,
            tc.tile_pool(name="sb2", bufs=2) as sb2,
            tc.tile_pool(name="ps_acc", bufs=1, space="PSUM") as ps_acc,
            tc.tile_pool(name="ps_tp", bufs=1, space="PSUM") as ps_tp,
            tc.tile_pool(name="ps2", bufs=1, space="PSUM") as ps2,
            tc.tile_pool(name="ps_e", bufs=2, space="PSUM") as ps_e,
            tc.tile_pool(name="dram", bufs=1, space="DRAM") as dram,
        ):
            # ---- constant loads (overlap with W1 streaming) ----
            xhi = cpool.tile([128, NK, B], BF)
            nc.gpsimd.dma_start(xhi[:], xhi_d[:])
            w2tb = cpool.tile([128, 2, FEAT], BF)
            nc.gpsimd.dma_start(w2tb[:],
                                w2tb_d.rearrange("(c p) f -> p c f", p=128))
            mw1tb = cpool.tile([128, 2, HID], BF)
            nc.gpsimd.dma_start(mw1tb[:],
                                mw1tb_d.rearrange("(c p) f -> p c f", p=128))
            dw2 = cpool.tile([128, 2, T * FEAT], F8)
            nc.gpsimd.dma_start(dw2[:],
                                dw2_d.rearrange("(c p) f -> p c f", p=128))
            mw2tb = cpool.tile([HID, T], BF)
            nc.gpsimd.dma_start(mw2tb[:], mw2tb_d[:])
            browb = cpool.tile([1, BROW_N], BF)
            nc.gpsimd.dma_start(browb[:], browb_d[:])
            biasz8 = cpool.tile([B, FEAT], F32)
            nc.gpsimd.dma_start(biasz8[:], biasz8_d[:])
            bias8u = cpool.tile([B, UCOLS], F32)
            nc.gpsimd.dma_start(bias8u[:], bias8u_d[:])
            id8 = cpool.tile([B, B], F32)
            nc.gpsimd.dma_start(id8[:], id8_d[:])
            db18 = cpool.tile([T, FEAT], BF)
            nc.gpsimd.dma_start(db18[:], db18_d[:])
            idsum = cpool.tile([N_CORES * B, B], F32)
            nc.gpsimd.dma_start(idsum[:], idsum_d[:])
            ones1b = cpool.tile([1, B], BF)
            nc.gpsimd.memset(ones1b[:], 1.0)

            # stream accumulation banks (live through the whole stream)
            bkzh = ps_acc.tile([128, 512], F32, tag="bkzh", name="bkzh")
            bku01 = ps_acc.tile([128, 512], F32, tag="bku01", name="bku01")
            bku23 = ps_acc.tile([128, 512], F32, tag="bku23", name="bku23")

            qs = (nc.scalar, nc.sync)
            w1env = (wbf_d, wpool, xhi, bkzh)
            w8env = (w8_d, wpool8, xhi, bku01, bku23)

            if reps1 > 1:
                # ---- bench build: stream only, no collectives/tail ----
                with tc.For_i(0, reps1, 1):
                    for _bi in range(body):
                        _emit_w1_stream(nc, tc, w1env, qs)
                        S_z = sb.tile([B, FEAT], F32, tag="S", name="S")
                        nc.vector.tensor_add(S_z[:], bkzh[0:B, 0:FEAT],
                                             biasz8[:])
                        for g in range(NGROUPS):
                            _emit_dw1_group(nc, tc, w8env, qs, g)
                        U = sb.tile([B, UCOLS], F32, tag="U", name="U")
                        nc.vector.scalar_tensor_tensor(
                            U[:, 0:512], bku01[64:64 + B, :], 1.0 / DW1_SCALE,
                            bias8u[:, 0:512], op0=AOT.mult, op1=AOT.add)
                        nc.vector.scalar_tensor_tensor(
                            U[:, 512:1024], bku23[96:96 + B, :],
                            1.0 / DW1_SCALE, bias8u[:, 512:1024],
                            op0=AOT.mult, op1=AOT.add)
                        nc.sync.dma_start(out_d[:], S_z[:])
            else:
                # ---- production build ----
                _emit_w1_stream(nc, tc, w1env, qs)

                # z partial + b1/8 -> AllReduce (hidden under dW1 stream)
                S_z = sb.tile([B, FEAT], F32, tag="S", name="S")
                nc.vector.tensor_add(S_z[:], bkzh[0:B, 0:FEAT], biasz8[:])
                cin_z = dram.tile([B, FEAT], F32, tag="cin_z", name="cin_z")
                cout_z = dram.tile([N_CORES * B, FEAT], F32, tag="cout_z",
                                   name="cout_z")
                nc.gpsimd.dma_start(cin_z[:], S_z[:])
                # AllGather (~17.5us dependent latency vs ~34.5us AllReduce)
                # + single PE matmul sum: keeps the whole z chain + tail
                # hidden under the fast dW1 stream.
                nc.gpsimd.collective_compute(
                    "AllGather", AOT.bypass,
                    replica_groups=[list(range(N_CORES))],
                    ins=[cin_z.opt()], outs=[cout_z.opt()],
                )
                Zg = sb.tile([N_CORES * B, FEAT], F32, tag="Zg", name="Zg")
                nc.gpsimd.dma_start(Zg[:], cout_z[:])
                prz = ps2.tile([B, FEAT], F32, tag="pp", name="prz")
                nc.tensor.matmul(prz[:], idsum[:], Zg[:],
                                 start=True, stop=True)
                R_z = sb.tile([B, FEAT], F32, tag="R", name="R")
                nc.vector.tensor_copy(R_z[:], prz[:])

                _emit_dw1_group(nc, tc, w8env, qs, 0)

                tail_env = (sb, sb2, ps_tp, ps2, ps_e, R_z, w2tb, mw1tb,
                            dw2, mw2tb, browb, id8, ones1b, db18)
                mask, cq, cb, q8 = _emit_tail(nc, tc, tail_env)

                for g in range(1, NGROUPS):
                    _emit_dw1_group(nc, tc, w8env, qs, g)

                # ---- endgame: local correction; host sums 8 partials ----
                ga = sb2.tile([B, FEAT], F32, tag="gacc", name="ga0")
                nc.vector.scalar_tensor_tensor(
                    ga[:], bku01[64:64 + B, 0:256], cq[:, 0:1], cb[:],
                    op0=AOT.mult, op1=AOT.add)
                ga2 = sb2.tile([B, FEAT], F32, tag="gacc", name="ga1")
                nc.vector.scalar_tensor_tensor(
                    ga2[:], bku01[64:64 + B, 256:512], cq[:, 1:2], ga[:],
                    op0=AOT.mult, op1=AOT.add)
                ga3 = sb2.tile([B, FEAT], F32, tag="gacc", name="ga2")
                nc.vector.scalar_tensor_tensor(
                    ga3[:], bku23[96:96 + B, 0:256], cq[:, 2:3], ga2[:],
                    op0=AOT.mult, op1=AOT.add)
                ga4 = sb2.tile([B, FEAT], F32, tag="gacc", name="ga3")
                nc.vector.scalar_tensor_tensor(
                    ga4[:], bku23[96:96 + B, 256:512], cq[:, 3:4], ga3[:],
                    op0=AOT.mult, op1=AOT.add)
                Gm = sb.tile([B, FEAT], F32, tag="Gm", name="Gm")
                nc.vector.tensor_mul(Gm[:], ga4[:], mask[:])

                gTb = sb.tile([128, 2, B], BF, tag="gTb", name="gTb")
                for c in range(2):
                    tp = ps_tp.tile([128, B], F32, tag="tp", name="tpg")
                    nc.tensor.transpose(tp[:], Gm[:, c * 128:(c + 1) * 128],
                                        id8[:])
                    nc.vector.tensor_copy(gTb[:, c, :], tp[:])
                pg = ps2.tile([B, FEAT], F32, tag="pp", name="pg")
                nc.tensor.matmul(pg[:], gTb[:, 0, :], w2tb[:, 0, :],
                                 start=True, stop=False)
                nc.tensor.matmul(pg[:], gTb[:, 1, :], w2tb[:, 1, :],
                                 start=False, stop=True)
                O = sb.tile([B, FEAT], F32, tag="O", name="O")
                nc.vector.tensor_add(O[:], pg[:], q8[:])
                nc.sync.dma_start(out_d[:], O[:])

    nc.compile()
    return nc


def _get_nc(reps1=1, body=1, wbufs=3):
    key = ("nc", reps1, body, wbufs)
    if key not in _CACHE:
        _CACHE[key] = _build(reps1, body, wbufs)
    return _CACHE[key]


def _prep_inputs(x, W1, b1, W2, b2, mW1, mb1, mW2, mb2, dW1, db1, dW2, db2):
    f32 = np.float32
    xflat = np.ascontiguousarray(np.asarray(x, f32).reshape(B, D_IN))
    W1 = np.asarray(W1, f32)
    W2 = np.asarray(W2, f32)
    dW1 = np.asarray(dW1, f32)
    dW2 = np.asarray(dW2, f32)
    mW1 = np.asarray(mW1, f32)
    mW2 = np.asarray(mW2, f32)
    b1 = np.asarray(b1, f32)
    b2 = np.asarray(b2, f32)
    db1 = np.asarray(db1, f32)
    db2 = np.asarray(db2, f32)
    mb1 = np.asarray(mb1, f32)
    mb2 = np.asarray(mb2, f32)

    # shared constants
    w2tb = np.ascontiguousarray(W2.T).astype(BF16)          # [g, f]
    mw1tb = np.ascontiguousarray(mW1.T).astype(BF16)        # [f, hid]
    dw2cat = np.ascontiguousarray(
        np.concatenate([dW2[t].T for t in range(T)], axis=1)
        * DW1_SCALE).astype(FP8)
    mw2tb = np.ascontiguousarray(mW2.T).astype(BF16)        # [hid, T]
    db2cat = np.concatenate([db2[t] for t in range(T)]) * DW1_SCALE
    browb = np.concatenate([b2, mb1, mb2, db2cat]).reshape(1, -1).astype(BF16)
    biasz8 = np.broadcast_to(b1 / N_CORES, (B, FEAT)).astype(f32).copy()
    bias8u = np.zeros((B, UCOLS), f32)
    for t in range(T):
        bias8u[:, 256 * t:256 * (t + 1)] = db1[t] / N_CORES
    id8 = np.eye(B, dtype=f32)
    db18 = (db1 / N_CORES).astype(BF16)                  # [T, FEAT]
    idsum = np.tile(np.eye(B, dtype=f32), (N_CORES, 1))  # [64, 8]

    def p_major(a, cols):
        # [KC, cols] -> [128, NK, cols] with k = c*128 + p
        return np.ascontiguousarray(
            a.reshape(NK, 128, cols).transpose(1, 0, 2))

    in_maps = []
    for c in range(N_CORES):
        sl = slice(c * KC, (c + 1) * KC)
        wbf = p_major(np.ascontiguousarray(W1[:, sl].T).astype(BF16), FEAT)
        w8f = np.empty((KC, UCOLS), dtype=FP8)
        for t in range(T):
            w8f[:, 256 * t:256 * (t + 1)] = (
                dW1[t, :, sl].T * DW1_SCALE).astype(FP8)
        w8 = p_major(w8f, UCOLS)

        xc = np.ascontiguousarray(xflat[:, sl].T)           # [KC, B]
        xh = xc.astype(BF16)

        in_maps.append({
            "wbf": wbf,
            "w8": w8,
            "xhi": p_major(xh, B),
            "w2tb": w2tb,
            "mw1tb": mw1tb,
            "dw2cat": dw2cat,
            "mw2tb": mw2tb,
            "browb": browb,
            "biasz8": biasz8,
            "bias8u": bias8u,
            "ident8": id8,
            "db18": db18,
            "idsum": idsum,
        })
    return in_maps


def run(trace=False, reps1=1, body=1, wbufs=3, **inputs):
    nc = _get_nc(reps1, body, wbufs)
    in_maps = _prep_inputs(**inputs)
    res = run_bass_kernel_spmd(nc, in_maps, core_ids=list(range(N_CORES)),
                               trace=trace)
    # unshard: per-core outputs are partial sums over the k shards
    out = np.sum(np.stack([r["out"] for r in res.results], 0),
                 axis=0, dtype=np.float64).astype(np.float32)
    return out, res


def kernel(**inputs) -> np.ndarray:
    import time as _time
    try:
        out, _ = run(trace=False, **inputs)
    except Exception:
        # transient device/runtime hiccups: retry once
        _time.sleep(3.0)
        out, _ = run(trace=False, **inputs)
    return out
